# revision 1
# baseline (speedup 1.0000x reference)
"""DGCNN encoder Bass kernel for 8 Trainium2 NeuronCores.

Hardcoded for B=2,T=2,N=256,K=27. Channel-split of the conv layers across
8 cores (L0 replicated); knn on-device (DVE top-8 rounds); neighbor
gather as one-hot expansion matmuls on TensorE with an exact bf16 hi/lo
split of U; max-over-K via DVE grouped reduce; BatchNorm stats gather-free
via adjacency-count matmuls; 2 AllGathers + 1 AllReduce.
"""
import sys
sys.path.insert(0, "/opt/trn_rl_repo")
import numpy as np

import concourse.bass as bass
import concourse.bacc as bacc
import concourse.mybir as mybir
import concourse.tile as tile
from concourse.bass_utils import run_bass_kernel_spmd

F32 = mybir.dt.float32
BF16 = mybir.dt.bfloat16
U16 = mybir.dt.uint16
AX = mybir.AxisListType.X
OP = mybir.AluOpType
AF = mybir.ActivationFunctionType

NCORE = 8
B, T, N, K = 2, 2, 256, 27
BT = B * T
PTS = BT * N          # 1024
NK = N * K            # 6912
EPS = 1e-5
COUT = [48, 192, 768, 3072]
FIN = [3, 96, 384, 1536]
SL = [48, 24, 96, 384]        # per-core out-channels (L0 replicated=full)
MBN = BT * N * K
CHK = 32                      # points per expansion chunk
NCHK = N // CHK


import os
ABLATE = set(os.environ.get("DGCNN_ABLATE", "").split(","))


def _cdiv(a, b):
    return (a + b - 1) // b


def build_nc():
    nc = bacc.Bacc("TRN2", target_bir_lowering=False, debug=False,
                   num_devices=NCORE)
    dt = nc.dram_tensor
    io = {}
    io["x_t"] = dt("x_t", [BT, 3, N], F32, kind="ExternalInput")
    io["iota"] = dt("iota", [256, 1], F32, kind="ExternalInput")
    io["negones"] = dt("negones", [3, 128], F32, kind="ExternalInput")
    for li in range(4):
        f = FIN[li]
        io[f"wa{li}"] = dt(f"wa{li}", [f, SL[li]], F32, kind="ExternalInput")
        io[f"wd{li}"] = dt(f"wd{li}", [f, SL[li]], F32, kind="ExternalInput")
        io[f"g{li}"] = dt(f"g{li}", [SL[li], 1], F32, kind="ExternalInput")
        io[f"b{li}"] = dt(f"b{li}", [SL[li], 1], F32, kind="ExternalInput")
    io["w4t"] = dt("w4t", [1104, 256], F32, kind="ExternalInput")
    io["g4"] = dt("g4", [256, 1], F32, kind="ExternalInput")
    io["b4"] = dt("b4", [256, 1], F32, kind="ExternalInput")
    out_d = dt("out", [B, T, 256, N], F32, kind="ExternalOutput")

    flatidx = dt("flatidx", [BT, NK], BF16)
    g_dram = dt("g_dram", [BT, 2, 128, NK], BF16)
    comb0_d = dt("comb0_d", [96, PTS], F32)
    ag1_in = dt("ag1_in", [2 * SL[1], PTS], F32)
    ag1_out = dt("ag1_out", [2 * COUT[1], PTS], F32, addr_space="Shared")
    ag2_in = dt("ag2_in", [2 * SL[2], PTS], F32)
    ag2_out = dt("ag2_out", [2 * COUT[2], PTS], F32, addr_space="Shared")
    comb3_d = dt("comb3_d", [2 * SL[3], PTS], F32)
    ar_in = dt("ar_in", [256, PTS], F32)
    ar_out = dt("ar_out", [256, PTS], F32, addr_space="Shared")
    rg = [list(range(NCORE))]

    with tile.TileContext(nc) as tc:
        class P:
            def __init__(self, pool):
                self.pool = pool

            def tile(self, shape, dtype, tag="t", bufs=None):
                return self.pool.tile(shape, dtype, tag=tag, name=tag,
                                      bufs=bufs)

            def release(self):
                self.pool.release()

        pers = P(tc.alloc_tile_pool(name="pers", bufs=1))
        work = P(tc.alloc_tile_pool(name="work", bufs=2))
        pe = P(tc.alloc_tile_pool(name="pe", bufs=2, space="PSUM"))
        pa = P(tc.alloc_tile_pool(name="pa", bufs=2, space="PSUM"))
        pb = P(tc.alloc_tile_pool(name="pb", bufs=2, space="PSUM"))

        v = nc.vector
        sc = nc.scalar
        te = nc.tensor
        gp = nc.gpsimd
        sy = nc.sync

        # ---- constants ----
        iota_f = []
        for t in range(2):
            it_f = pers.tile([128, 1], F32, tag=f"iotaf{t}")
            sy.dma_start(it_f[:], io["iota"].ap()[t * 128:(t + 1) * 128])
            iota_f.append(it_f)
        negones = pers.tile([3, 128], F32, tag="negones")
        sy.dma_start(negones[:], io["negones"].ap())
        eps_col = pers.tile([128, 1], F32, tag="eps_col")
        v.memset(eps_col[:], EPS)
        zero_col = pers.tile([128, 1], F32, tag="zero_col")
        v.memset(zero_col[:], 0.0)
        xf = pers.tile([3, PTS], F32, tag="xf")
        for f in range(BT):
            sy.dma_start(xf[:, f * N:(f + 1) * N], io["x_t"].ap()[f])

        # ---- knn (replicated): top-27 neighbor indices per frame ----
        for f in range(BT):
            xt = xf[:, f * N:(f + 1) * N]
            xsqh = work.tile([3, N], F32, tag="xsqh")
            v.scalar_tensor_tensor(xsqh[:], xt, 0.5, xt, OP.mult, OP.mult)
            for t in range(2):
                ps = pa.tile([128, N], F32, tag="pa")
                te.matmul(ps[:], xt[:, t * 128:(t + 1) * 128], xt,
                          start=True, stop=False)
                te.matmul(ps[:], negones[:], xsqh[:], start=False, stop=True)
                s_a = work.tile([128, N], F32, tag="sc_a")
                v.tensor_copy(s_a[:], ps[:])
                s_b = work.tile([128, N], F32, tag="sc_b")
                idx32 = work.tile([128, 32], U16, tag="idx32")
                cur, nxt = s_a, s_b
                if "knn" in ABLATE:
                    v.memset(idx32[:], 0)
                for r in range(4 if "knn" not in ABLATE else 0):
                    m8 = work.tile([128, 8], F32, tag="m8")
                    v.max(m8[:], cur[:])
                    v.max_index(idx32[:, 8 * r:8 * r + 8], m8[:], cur[:])
                    if r < 3:
                        v.match_replace(nxt[:], m8[:], cur[:], -3.0e38)
                        cur, nxt = nxt, cur
                idxf = work.tile([128, K], F32, tag="idxf")
                v.tensor_copy(idxf[:], idx32[:, :K])
                idxb = work.tile([128, K], BF16, tag="idxb")
                v.tensor_copy(idxb[:], idxf[:])
                dst = flatidx.ap()[f].rearrange("(n k) -> n k", k=K)
                sy.dma_start(dst[t * 128:(t + 1) * 128], idxb[:])

        def build_G(f, first):
            gts = []
            if first:
                bc = work.tile([128, NK], BF16, tag="g", bufs=3)
                sy.dma_start(bc[0:1, :], flatidx.ap()[f:f + 1, :])
                s = 1
                while s < 128:
                    sy.dma_start(bc[s:2 * s, :], bc[0:s, :])
                    s *= 2
                for t in range(2):
                    gt = work.tile([128, NK], BF16, tag="g", bufs=3)
                    v.tensor_scalar(gt[:], bc[:], iota_f[t][:], None,
                                    OP.is_equal)
                    sy.dma_start(g_dram.ap()[f, t], gt[:])
                    gts.append(gt)
            else:
                for t in range(2):
                    gt = work.tile([128, NK], BF16, tag="g", bufs=3)
                    for c8 in range(NCHK):
                        cl = c8 * CHK * K
                        ch_ = (c8 + 1) * CHK * K
                        sy.dma_start(gt[:, cl:ch_],
                                     g_dram.ap()[f, t, :, cl:ch_])
                    gts.append(gt)
            return gts

        A4 = [None] * BT
        CNT = [[None, None] for _ in range(BT)]

        for li in range(4):
            C8 = SL[li]
            F = FIN[li]
            NCT = _cdiv(C8, 128)
            # contract blocks (f32)
            if li == 0:
                fblocks = [(xf, 3)]
            elif li == 1:
                fb0 = work.tile([48, PTS], F32, tag="fb0", bufs=1)
                sy.dma_start(fb0[:], comb0_d.ap()[0:48])
                fb1 = work.tile([48, PTS], F32, tag="fb1", bufs=1)
                sy.dma_start(fb1[:], comb0_d.ap()[48:96])
                fblocks = [(fb0, 48), (fb1, 48)]
            else:
                ag = ag1_out if li == 2 else ag2_out
                fblocks = []
                for kb in range(F // 128):
                    fb = work.tile([128, PTS], F32, tag=f"fb{kb}", bufs=1)
                    sy.dma_start(fb[:], ag.ap()[kb * 128:(kb + 1) * 128])
                    fblocks.append((fb, 128))
            # wa resident
            wa_blk = []
            ro = 0
            for bi, (fb, rows) in enumerate(fblocks):
                wa = work.tile([rows, 384], F32, tag=f"wab{bi}",
                               bufs=1)[:, :C8]
                sy.dma_start(wa[:], io[f"wa{li}"].ap()[ro:ro + rows])
                wa_blk.append(wa)
                ro += rows

            # U^T per pts-tile -> exact bf16 hi/lo pairs (plus squares)
            s2acc = []
            if li > 0:
                for ct in range(NCT):
                    cw = min(128, C8 - ct * 128)
                    sa = work.tile([cw, 1], F32, tag=f"s2a{ct}", bufs=1)
                    v.memset(sa[:], 0.0)
                    s2acc.append(sa)
            uh, ul, u2h, u2l = [], [], [], []
            for pt in range(8):
                if "uv" in ABLATE:
                    for lst, tg in ((uh, "uh"), (ul, "ul"), (u2h, "u2h"),
                                    (u2l, "u2l")):
                        t_ = work.tile([128, 384], BF16, tag=f"{tg}{pt}",
                                       bufs=1)[:, :C8]
                        v.memset(t_[:], 0)
                        lst.append(t_)
                    continue
                pu = pb.tile([128, C8], F32, tag="pb")
                for bi, (fb, rows) in enumerate(fblocks):
                    te.matmul(pu[:], fb[:, pt * 128:(pt + 1) * 128],
                              wa_blk[bi][:], start=(bi == 0),
                              stop=(bi == len(fblocks) - 1))
                h_ = work.tile([128, 384], BF16, tag=f"uh{pt}",
                               bufs=1)[:, :C8]
                sc.copy(h_[:], pu[:])
                l_ = work.tile([128, 384], BF16, tag=f"ul{pt}",
                               bufs=1)[:, :C8]
                v.scalar_tensor_tensor(l_[:], pu[:], 1.0, h_[:],
                                       OP.mult, OP.subtract)
                sqf = work.tile([128, 384], F32, tag="sqf")[:, :C8]
                sc.square(sqf[:], pu[:])
                if li == 0:
                    h2 = work.tile([128, 384], BF16, tag=f"u2h{pt}",
                                   bufs=1)[:, :C8]
                    sc.copy(h2[:], sqf[:])
                    l2 = work.tile([128, 384], BF16, tag=f"u2l{pt}",
                                   bufs=1)[:, :C8]
                    v.scalar_tensor_tensor(l2[:], sqf[:], 1.0, h2[:],
                                           OP.mult, OP.subtract)
                    u2h.append(h2)
                    u2l.append(l2)
                else:
                    # total-only S2: sum_m cnt(m)*U^2(c,m), 1-col matmuls
                    for ct in range(NCT):
                        cw = min(128, C8 - ct * 128)
                        cs = ct * 128
                        p2 = pa.tile([cw, 512], F32, tag="pa")[:, :1]
                        te.matmul(p2[:], sqf[:, cs:cs + cw],
                                  CNT[pt // 2][pt % 2][:],
                                  start=True, stop=True)
                        v.tensor_tensor(s2acc[ct][:], s2acc[ct][:], p2[:],
                                        OP.add)
                uh.append(h_)
                ul.append(l_)

            # V (f32), wd streamed
            V = []
            for ct in range(NCT):
                cw = min(128, C8 - ct * 128)
                vt = work.tile([cw, PTS], F32, tag=f"V{ct}", bufs=1)
                if "v" in ABLATE:
                    v.memset(vt[:], 0)
                    V.append(vt)
                    continue
                for ch in range(2):
                    pv = pa.tile([cw, 512], F32, tag="pa")
                    ro = 0
                    for bi, (fb, rows) in enumerate(fblocks):
                        wd = work.tile([rows, 128], F32, tag="wdblk",
                                       bufs=3)[:, :cw]
                        sy.dma_start(wd[:], io[f"wd{li}"].ap()[
                            ro:ro + rows, ct * 128:ct * 128 + cw])
                        te.matmul(pv[:], wd[:],
                                  fb[:, ch * 512:(ch + 1) * 512],
                                  start=(bi == 0),
                                  stop=(bi == len(fblocks) - 1))
                        ro += rows
                    sc.copy(vt[:, ch * 512:(ch + 1) * 512], pv[:])
                V.append(vt)

            racc_s = [work.tile([min(128, C8 - ct * 128), BT], F32,
                                tag=f"rs{ct}") for ct in range(NCT)]
            racc_s2 = [work.tile([min(128, C8 - ct * 128), BT], F32,
                                 tag=f"rs2{ct}") for ct in range(NCT)]
            racc_vs = [work.tile([min(128, C8 - ct * 128), BT], F32,
                                 tag=f"rvs{ct}") for ct in range(NCT)]
            upool = [work.tile([min(128, C8 - ct * 128), PTS], F32,
                               tag=f"up{ct}", bufs=1) for ct in range(NCT)]
            if "pool" in ABLATE:
                for u_ in upool:
                    v.memset(u_[:], 0)
            if "stats" in ABLATE:
                for r_ in racc_s + racc_s2 + racc_vs:
                    v.memset(r_[:], 0)

            for f in range(BT):
                gts = build_G(f, li == 0) if "g" not in ABLATE else None
                if li == 0 and "g" not in ABLATE:
                    ats = []
                    with nc.allow_low_precision(reason="0/1 adjacency"):
                        for t in range(2):
                            at = pers.tile([128, N], BF16, tag=f"A{f}_{t}")
                            v.tensor_reduce(at[:], gts[t][:].rearrange(
                                "p (n k) -> p n k", k=K), axis=AX, op=OP.add)
                            ats.append(at)
                    A4[f] = ats
                    for t in range(2):
                        cn = pers.tile([128, 1], F32, tag=f"cnt{f}_{t}")
                        v.tensor_reduce(cn[:], A4[f][t][:], axis=AX,
                                        op=OP.add)
                        CNT[f][t] = cn
                for ct in range(NCT):
                    cw = min(128, C8 - ct * 128)
                    cs = ct * 128
                    for c8 in range(NCHK if "exp" not in ABLATE else 0):
                        px = pe.tile([cw, CHK * K], F32, tag="pe")
                        for sub0, sub1 in ((0, 512), (512, CHK * K)):
                            ops = [(0, uh), (0, ul), (1, uh), (1, ul)]
                            for oi, (mt, usrc) in enumerate(ops):
                                te.matmul(
                                    px[:, sub0:sub1],
                                    usrc[2 * f + mt][:, cs:cs + cw],
                                    gts[mt][:, c8 * CHK * K + sub0:
                                            c8 * CHK * K + sub1],
                                    start=(oi == 0), stop=(oi == 3))
                        if "pool" not in ABLATE:
                            v.tensor_reduce(
                                upool[ct][:, f * N + c8 * CHK:
                                          f * N + (c8 + 1) * CHK],
                                px[:].rearrange("p (n k) -> p n k", k=K),
                                axis=AX, op=OP.max)
                    if "stats" in ABLATE:
                        continue
                    ps1 = pa.tile([cw, N], F32, tag="pa")
                    ops = [(0, uh), (0, ul), (1, uh), (1, ul)]
                    for oi, (mt, usrc) in enumerate(ops):
                        te.matmul(ps1[:], usrc[2 * f + mt][:, cs:cs + cw],
                                  A4[f][mt][:], start=(oi == 0),
                                  stop=(oi == 3))
                    v.tensor_reduce(racc_s[ct][:, f:f + 1], ps1[:],
                                    axis=AX, op=OP.add)
                    scr = work.tile([cw, N], F32, tag="scr")
                    v.scalar_tensor_tensor(
                        scr[:], V[ct][:, f * N:(f + 1) * N], 1.0, ps1[:],
                        OP.mult, OP.mult, accum_out=racc_vs[ct][:, f:f + 1])
                    if li == 0:
                        ps2 = pa.tile([cw, N], F32, tag="pa")
                        ops = [(0, u2h), (0, u2l), (1, u2h), (1, u2l)]
                        for oi, (mt, usrc) in enumerate(ops):
                            te.matmul(ps2[:],
                                      usrc[2 * f + mt][:, cs:cs + cw],
                                      A4[f][mt][:], start=(oi == 0),
                                      stop=(oi == 3))
                        v.tensor_reduce(racc_s2[ct][:, f:f + 1], ps2[:],
                                        axis=AX, op=OP.add)

            # finalize stats + BN + lrelu + tp; write comb to DRAM
            for ct in range(NCT):
                cw = min(128, C8 - ct * 128)
                cs = ct * 128
                col = lambda tag: work.tile([cw, 1], F32, tag=tag)
                rs, rs2, rvs = col("c_rs"), col("c_rs2"), col("c_rvs")
                v.tensor_reduce(rs[:], racc_s[ct][:], axis=AX, op=OP.add)
                if li == 0:
                    v.tensor_reduce(rs2[:], racc_s2[ct][:], axis=AX,
                                    op=OP.add)
                else:
                    rs2 = s2acc[ct]
                v.tensor_reduce(rvs[:], racc_vs[ct][:], axis=AX, op=OP.add)
                rv, rv2 = col("c_rv"), col("c_rv2")
                scrv = work.tile([cw, PTS], F32, tag="hwork")
                v.tensor_scalar(scrv[:], V[ct][:], 1.0, 0.0, OP.mult,
                                OP.add, accum_out=rv[:])
                v.scalar_tensor_tensor(scrv[:], V[ct][:], 1.0, V[ct][:],
                                       OP.mult, OP.mult, accum_out=rv2[:])
                sum_h, t1, sum_h2 = col("c_sh"), col("c_t1"), col("c_sh2")
                v.scalar_tensor_tensor(sum_h[:], rv[:], float(K), rs[:],
                                       OP.mult, OP.add)
                v.scalar_tensor_tensor(t1[:], rvs[:], 2.0, rs2[:],
                                       OP.mult, OP.add)
                v.scalar_tensor_tensor(sum_h2[:], rv2[:], float(K), t1[:],
                                       OP.mult, OP.add)
                mean, ex2, var = col("c_mean"), col("c_ex2"), col("c_var")
                v.tensor_scalar(mean[:], sum_h[:], 1.0 / MBN, None, OP.mult)
                v.tensor_scalar(ex2[:], sum_h2[:], 1.0 / MBN, None, OP.mult)
                m2 = col("c_m2")
                v.tensor_tensor(m2[:], mean[:], mean[:], OP.mult)
                v.tensor_tensor(var[:], ex2[:], m2[:], OP.subtract)
                std, inv, scal, bias2 = (col("c_std"), col("c_inv"),
                                         col("c_scal"), col("c_bias"))
                sc.activation(std[:], var[:], AF.Sqrt, bias=eps_col[:cw])
                v.reciprocal(inv[:], std[:])
                gcol = work.tile([cw, 1], F32, tag="gcol")
                sy.dma_start(gcol[:], io[f"g{li}"].ap()[cs:cs + cw])
                bcol = work.tile([cw, 1], F32, tag="bcol")
                sy.dma_start(bcol[:], io[f"b{li}"].ap()[cs:cs + cw])
                v.tensor_tensor(scal[:], inv[:], gcol[:], OP.mult)
                tm = col("c_tm")
                v.tensor_tensor(tm[:], mean[:], scal[:], OP.mult)
                v.tensor_tensor(bias2[:], bcol[:], tm[:], OP.subtract)
                h = work.tile([cw, PTS], F32, tag="hwork")
                v.tensor_tensor(h[:], upool[ct][:], V[ct][:], OP.add)
                bn = work.tile([cw, PTS], F32, tag="hwork")
                v.tensor_scalar(bn[:], h[:], scal[:], bias2[:],
                                OP.mult, OP.add)
                hpost = work.tile([cw, PTS], F32, tag="hwork")
                v.scalar_tensor_tensor(hpost[:], bn[:], 0.2, bn[:],
                                       OP.mult, OP.max)
                tps = work.tile([cw, N], F32, tag="tps")
                v.tensor_tensor(tps[:], hpost[:, 0:N], hpost[:, N:2 * N],
                                OP.add)
                tpx = work.tile([cw, N], F32, tag="tpx")
                v.tensor_scalar(tpx[:], tps[:], 0.5, None, OP.mult)
                # write [hp; tp] to the layer's comb DRAM buffer
                cd = (comb0_d, ag1_in, ag2_in, comb3_d)[li]
                sy.dma_start(cd.ap()[cs:cs + cw], hpost[:])
                for f in range(BT):
                    sy.dma_start(
                        cd.ap()[C8 + cs:C8 + cs + cw,
                                f * N:(f + 1) * N], tpx[:])
            if li == 1:
                gp.collective_compute("AllGather", OP.bypass,
                                      replica_groups=rg,
                                      ins=[ag1_in.ap()], outs=[ag1_out.ap()])
            elif li == 2:
                gp.collective_compute("AllGather", OP.bypass,
                                      replica_groups=rg,
                                      ins=[ag2_in.ap()], outs=[ag2_out.ap()])

        # ---- L4 ----
        pieces = [(comb0_d, 0, 48), (comb0_d, 48, 48),
                  (ag1_in, 0, 24), (ag1_in, 24, 24),
                  (ag2_in, 0, 96), (ag2_in, 96, 96)]
        for ct in range(3):
            pieces.append((comb3_d, ct * 128, 128))
        for ct in range(3):
            pieces.append((comb3_d, 384 + ct * 128, 128))

        for ch in range(2):
            if "l4" in ABLATE:
                for half in range(2):
                    yp = work.tile([128, 512], F32, tag="ypc")
                    v.memset(yp[:], 0)
                    sy.dma_start(ar_in.ap()[half * 128:(half + 1) * 128,
                                            ch * 512:(ch + 1) * 512], yp[:])
                continue
            pys = [pa.tile([128, 512], F32, tag="pa") for _ in range(2)]
            ro = 0
            for bi, (src, off, rows) in enumerate(pieces):
                pc = work.tile([128, 512], F32, tag="l4p", bufs=3)[:rows, :]
                sy.dma_start(pc[:], src.ap()[off:off + rows,
                                             ch * 512:(ch + 1) * 512])
                wb = work.tile([128, 256], F32, tag="l4w", bufs=3)[:rows, :]
                sy.dma_start(wb[:], io["w4t"].ap()[ro:ro + rows])
                for half in range(2):
                    te.matmul(pys[half][:],
                              wb[:, half * 128:(half + 1) * 128], pc[:],
                              start=(bi == 0), stop=(bi == len(pieces) - 1))
                ro += rows
            for half in range(2):
                yp = work.tile([128, 512], F32, tag="ypc")
                v.tensor_copy(yp[:], pys[half][:])
                sy.dma_start(ar_in.ap()[half * 128:(half + 1) * 128,
                                        ch * 512:(ch + 1) * 512], yp[:])
        gp.collective_compute("AllReduce", OP.add, replica_groups=rg,
                              ins=[ar_in.ap()], outs=[ar_out.ap()])
        for half in range(2):
            yf = work.tile([128, PTS], F32, tag="hwork")
            sy.dma_start(yf[:], ar_out.ap()[half * 128:(half + 1) * 128])
            col = lambda tag: work.tile([128, 1], F32, tag=tag)
            ry, ry2 = col("y_r"), col("y_r2")
            scr = work.tile([128, PTS], F32, tag="hwork")
            v.tensor_scalar(scr[:], yf[:], 1.0, 0.0, OP.mult, OP.add,
                            accum_out=ry[:])
            v.scalar_tensor_tensor(scr[:], yf[:], 1.0, yf[:], OP.mult,
                                   OP.mult, accum_out=ry2[:])
            mean, ex2, var = col("y_mean"), col("y_ex2"), col("y_var")
            v.tensor_scalar(mean[:], ry[:], 1.0 / PTS, None, OP.mult)
            v.tensor_scalar(ex2[:], ry2[:], 1.0 / PTS, None, OP.mult)
            m2 = col("y_m2")
            v.tensor_tensor(m2[:], mean[:], mean[:], OP.mult)
            v.tensor_tensor(var[:], ex2[:], m2[:], OP.subtract)
            std, inv, scal, bias2 = (col("y_std"), col("y_inv"),
                                     col("y_scal"), col("y_bias"))
            sc.activation(std[:], var[:], AF.Sqrt, bias=eps_col[:])
            v.reciprocal(inv[:], std[:])
            gcol = work.tile([128, 1], F32, tag="y_g")
            sy.dma_start(gcol[:], io["g4"].ap()[half * 128:(half + 1) * 128])
            bcol = work.tile([128, 1], F32, tag="y_b")
            sy.dma_start(bcol[:], io["b4"].ap()[half * 128:(half + 1) * 128])
            v.tensor_tensor(scal[:], inv[:], gcol[:], OP.mult)
            tm = col("y_tm")
            v.tensor_tensor(tm[:], mean[:], scal[:], OP.mult)
            v.tensor_tensor(bias2[:], bcol[:], tm[:], OP.subtract)
            bn = work.tile([128, PTS], F32, tag="hwork")
            v.tensor_scalar(bn[:], yf[:], scal[:], bias2[:], OP.mult, OP.add)
            yo = work.tile([128, PTS], F32, tag="hwork")
            v.scalar_tensor_tensor(yo[:], bn[:], 0.2, bn[:], OP.mult, OP.max)
            dst = out_d.ap().rearrange("b t c n -> c (b t) n")
            sy.dma_start(dst[half * 128:(half + 1) * 128],
                         yo[:].rearrange("p (f n) -> p f n", n=N))

        for p in (pb, pa, pe, work, pers):
            p.release()

    nc.compile()
    return nc


def _perm_for(C8):
    C = C8 * NCORE
    out = []
    for r in range(NCORE):
        out += list(range(r * C8, (r + 1) * C8))
        out += list(range(C + r * C8, C + (r + 1) * C8))
    return np.array(out)


def _prep_inputs(inputs):
    x = np.asarray(inputs["x"], np.float32)
    flat = x.reshape(BT, N, 3)
    x_t = np.ascontiguousarray(flat.transpose(0, 2, 1))
    iota = np.arange(256, dtype=np.float32).reshape(256, 1)
    negones = np.full((3, 128), -1.0, np.float32)

    base = {
        "x_t": x_t, "iota": iota, "negones": negones,
        "g4": np.asarray(inputs["g4"], np.float32).reshape(256, 1),
        "b4": np.asarray(inputs["b4"], np.float32).reshape(256, 1),
    }
    perm = np.arange(3)
    in_maps = [dict(base) for _ in range(NCORE)]
    w4 = np.asarray(inputs["w4"], np.float32)
    for li in range(4):
        C = COUT[li]
        F = FIN[li]
        W = np.asarray(inputs[f"w{li}"], np.float32)
        Wa_full = W[:, :F][:, perm]
        Wd_full = (W[:, F:] - W[:, :F])[:, perm]
        g = np.asarray(inputs[f"g{li}"], np.float32)
        b = np.asarray(inputs[f"b{li}"], np.float32)
        for r in range(NCORE):
            if li == 0:
                rows = slice(0, 48)
            else:
                rows = slice(r * SL[li], (r + 1) * SL[li])
            in_maps[r][f"wa{li}"] = np.ascontiguousarray(Wa_full[rows].T)
            in_maps[r][f"wd{li}"] = np.ascontiguousarray(Wd_full[rows].T)
            in_maps[r][f"g{li}"] = g[rows].reshape(-1, 1).copy()
            in_maps[r][f"b{li}"] = b[rows].reshape(-1, 1).copy()
        # L0 replicated: f1 = [hp0; tp0] stays in reference order.
        perm = np.arange(2 * C) if li == 0 else _perm_for(C // NCORE)
    for r in range(NCORE):
        cols = []
        cols.append((np.arange(0, 48), 1 / 8))
        cols.append((np.arange(48, 96), 1 / 8))
        cols.append((96 + np.arange(r * 24, (r + 1) * 24), 1.0))
        cols.append((288 + np.arange(r * 24, (r + 1) * 24), 1.0))
        cols.append((480 + np.arange(r * 96, (r + 1) * 96), 1.0))
        cols.append((1248 + np.arange(r * 96, (r + 1) * 96), 1.0))
        cols.append((2016 + np.arange(r * 384, (r + 1) * 384), 1.0))
        cols.append((5088 + np.arange(r * 384, (r + 1) * 384), 1.0))
        blocks = [np.ascontiguousarray(w4[:, c].T) * s for c, s in cols]
        in_maps[r]["w4t"] = np.concatenate(blocks, axis=0).astype(np.float32)
    return in_maps


_NC_CACHE = []


def kernel(**inputs):
    if not _NC_CACHE:
        _NC_CACHE.append(build_nc())
    nc = _NC_CACHE[0]
    in_maps = _prep_inputs(inputs)
    res = run_bass_kernel_spmd(nc, in_maps, list(range(NCORE)))
    return np.asarray(res.results[0]["out"])



# revision 7
# speedup vs baseline: 1.0814x; 1.0814x over previous
"""DGCNN encoder Bass kernel for 8 Trainium2 NeuronCores.

Hardcoded for B=2,T=2,N=256,K=27. Channel-split of the conv layers across
8 cores (L0 replicated); knn on-device (DVE top-8 rounds); neighbor
gather as one-hot expansion matmuls on TensorE with an exact bf16 hi/lo
split of U; max-over-K via DVE grouped reduce; BatchNorm stats gather-free
via adjacency-count matmuls; 2 AllGathers + 1 AllReduce.
"""
import sys
sys.path.insert(0, "/opt/trn_rl_repo")
import numpy as np

import concourse.bass as bass
import concourse.bacc as bacc
import concourse.mybir as mybir
import concourse.tile as tile
from concourse.bass_utils import run_bass_kernel_spmd

F32 = mybir.dt.float32
F32R = mybir.dt.float32r
BF16 = mybir.dt.bfloat16
U16 = mybir.dt.uint16
AX = mybir.AxisListType.X
OP = mybir.AluOpType
AF = mybir.ActivationFunctionType

NCORE = 8
B, T, N, K = 2, 2, 256, 27
BT = B * T
PTS = BT * N          # 1024
NK = N * K            # 6912
EPS = 1e-5
COUT = [48, 192, 768, 3072]
FIN = [3, 96, 384, 1536]
SL = [48, 24, 96, 384]        # per-core out-channels (L0 replicated=full)
MBN = BT * N * K
CHK = 32                      # points per expansion chunk
NCHK = N // CHK


import os
ABLATE = set(os.environ.get("DGCNN_ABLATE", "").split(","))


def _cdiv(a, b):
    return (a + b - 1) // b


def build_nc():
    nc = bacc.Bacc("TRN2", target_bir_lowering=False, debug=False,
                   num_devices=NCORE)
    dt = nc.dram_tensor
    io = {}
    io["x_t"] = dt("x_t", [BT, 3, N], F32, kind="ExternalInput")
    io["iota"] = dt("iota", [256, 1], F32, kind="ExternalInput")
    io["negones"] = dt("negones", [3, 128], F32, kind="ExternalInput")
    for li in range(4):
        f = FIN[li]
        io[f"wa{li}"] = dt(f"wa{li}", [f, SL[li]], F32, kind="ExternalInput")
        io[f"wd{li}"] = dt(f"wd{li}", [f, SL[li]], F32, kind="ExternalInput")
        io[f"g{li}"] = dt(f"g{li}", [SL[li], 1], F32, kind="ExternalInput")
        io[f"b{li}"] = dt(f"b{li}", [SL[li], 1], F32, kind="ExternalInput")
    io["w4t"] = dt("w4t", [1104, 256], F32, kind="ExternalInput")
    io["g4"] = dt("g4", [256, 1], F32, kind="ExternalInput")
    io["b4"] = dt("b4", [256, 1], F32, kind="ExternalInput")
    out_d = dt("out", [B, T, 256, N], F32, kind="ExternalOutput")

    flatidx = dt("flatidx", [BT, NK], BF16)
    g_dram = dt("g_dram", [BT, 2, 128, NK], BF16)
    comb0_d = dt("comb0_d", [96, PTS], F32)
    ag1_in = dt("ag1_in", [2 * SL[1], PTS], F32)
    ag1_out = dt("ag1_out", [2 * COUT[1], PTS], F32, addr_space="Shared")
    ag2_in = dt("ag2_in", [2 * SL[2], PTS], F32)
    ag2_out = dt("ag2_out", [2 * COUT[2], PTS], F32, addr_space="Shared")
    comb3_d = dt("comb3_d", [2 * SL[3], PTS], F32)
    ar_in = dt("ar_in", [256, PTS], F32)
    ar_out = dt("ar_out", [256, PTS], F32, addr_space="Shared")
    rg = [list(range(NCORE))]

    with tile.TileContext(nc) as tc:
        class P:
            def __init__(self, pool):
                self.pool = pool

            def tile(self, shape, dtype, tag="t", bufs=None):
                return self.pool.tile(shape, dtype, tag=tag, name=tag,
                                      bufs=bufs)

            def release(self):
                self.pool.release()

        pers = P(tc.alloc_tile_pool(name="pers", bufs=1))
        work = P(tc.alloc_tile_pool(name="work", bufs=2))
        pe = P(tc.alloc_tile_pool(name="pe", bufs=2, space="PSUM"))
        pa = P(tc.alloc_tile_pool(name="pa", bufs=2, space="PSUM"))
        pb = P(tc.alloc_tile_pool(name="pb", bufs=2, space="PSUM"))

        v = nc.vector
        sc = nc.scalar
        te = nc.tensor
        gp = nc.gpsimd
        sy = nc.sync

        # ---- constants ----
        iota_f = []
        for t in range(2):
            it_f = pers.tile([128, 1], F32, tag=f"iotaf{t}")
            sy.dma_start(it_f[:], io["iota"].ap()[t * 128:(t + 1) * 128])
            iota_f.append(it_f)
        negones = pers.tile([3, 128], F32, tag="negones")
        sy.dma_start(negones[:], io["negones"].ap())
        eps_col = pers.tile([128, 1], F32, tag="eps_col")
        v.memset(eps_col[:], EPS)
        zero_col = pers.tile([128, 1], F32, tag="zero_col")
        v.memset(zero_col[:], 0.0)
        xf = pers.tile([3, PTS], F32, tag="xf")
        for f in range(BT):
            sy.dma_start(xf[:, f * N:(f + 1) * N], io["x_t"].ap()[f])

        # ---- knn (replicated): top-27 neighbor indices per frame ----
        for f in range(BT):
            xt = xf[:, f * N:(f + 1) * N]
            xsqh = work.tile([3, N], F32, tag="xsqh")
            v.scalar_tensor_tensor(xsqh[:], xt, 0.5, xt, OP.mult, OP.mult)
            for t in range(2):
                ps = pa.tile([128, N], F32, tag="pa")
                te.matmul(ps[:], xt[:, t * 128:(t + 1) * 128], xt,
                          start=True, stop=False)
                te.matmul(ps[:], negones[:], xsqh[:], start=False, stop=True)
                s_a = work.tile([128, N], F32, tag="sc_a")
                v.tensor_copy(s_a[:], ps[:])
                s_b = work.tile([128, N], F32, tag="sc_b")
                idx32 = work.tile([128, 32], U16, tag="idx32")
                cur, nxt = s_a, s_b
                if "knn" in ABLATE:
                    v.memset(idx32[:], 0)
                for r in range(4 if "knn" not in ABLATE else 0):
                    m8 = work.tile([128, 8], F32, tag="m8")
                    v.max(m8[:], cur[:])
                    v.max_index(idx32[:, 8 * r:8 * r + 8], m8[:], cur[:])
                    if r < 3:
                        v.match_replace(nxt[:], m8[:], cur[:], -3.0e38)
                        cur, nxt = nxt, cur
                idxf = work.tile([128, K], F32, tag="idxf")
                v.tensor_copy(idxf[:], idx32[:, :K])
                idxb = work.tile([128, K], BF16, tag="idxb")
                v.tensor_copy(idxb[:], idxf[:])
                dst = flatidx.ap()[f].rearrange("(n k) -> n k", k=K)
                sy.dma_start(dst[t * 128:(t + 1) * 128], idxb[:])

        def build_G(f, first):
            gts = []
            if first:
                bc = work.tile([128, NK], BF16, tag="g", bufs=3)
                sy.dma_start(bc[0:1, :], flatidx.ap()[f:f + 1, :])
                s = 1
                while s < 128:
                    sy.dma_start(bc[s:2 * s, :], bc[0:s, :])
                    s *= 2
                for t in range(2):
                    gt = work.tile([128, NK], BF16, tag="g", bufs=3)
                    v.tensor_scalar(gt[:], bc[:], iota_f[t][:], None,
                                    OP.is_equal)
                    sy.dma_start(g_dram.ap()[f, t], gt[:])
                    gts.append(gt)
            else:
                for t in range(2):
                    gt = work.tile([128, NK], BF16, tag="g", bufs=3)
                    for c8 in range(NCHK):
                        cl = c8 * CHK * K
                        ch_ = (c8 + 1) * CHK * K
                        sy.dma_start(gt[:, cl:ch_],
                                     g_dram.ap()[f, t, :, cl:ch_])
                    gts.append(gt)
            return gts

        A4 = [None] * BT
        CNT = [[None, None] for _ in range(BT)]

        for li in range(4):
            C8 = SL[li]
            F = FIN[li]
            NCT = _cdiv(C8, 128)
            # contract blocks (f32)
            if li == 0:
                fblocks = [(xf, 3)]
            elif li == 1:
                fb0 = work.tile([48, PTS], F32, tag="fb0", bufs=1)
                sy.dma_start(fb0[:], comb0_d.ap()[0:48])
                fb1 = work.tile([48, PTS], F32, tag="fb1", bufs=1)
                sy.dma_start(fb1[:], comb0_d.ap()[48:96])
                fblocks = [(fb0, 48), (fb1, 48)]
            else:
                ag = ag1_out if li == 2 else ag2_out
                fblocks = []
                for kb in range(F // 128):
                    fb = work.tile([128, PTS], F32, tag=f"fb{kb}", bufs=1)
                    sy.dma_start(fb[:], ag.ap()[kb * 128:(kb + 1) * 128])
                    fblocks.append((fb, 128))
            # wa resident
            wa_blk = []
            ro = 0
            for bi, (fb, rows) in enumerate(fblocks):
                wa = work.tile([rows, 384], F32, tag=f"wab{bi}",
                               bufs=1)[:, :C8]
                sy.dma_start(wa[:], io[f"wa{li}"].ap()[ro:ro + rows])
                wa_blk.append(wa)
                ro += rows

            # U^T per pts-tile -> exact bf16 hi/lo pairs (plus squares)
            s2acc = []
            if li > 0:
                for ct in range(NCT):
                    cw = min(128, C8 - ct * 128)
                    sa = work.tile([cw, 1], F32, tag=f"s2a{ct}", bufs=1)
                    v.memset(sa[:], 0.0)
                    s2acc.append(sa)
            uh, ul, u2h, u2l = [], [], [], []
            for pt in range(8):
                if "uv" in ABLATE:
                    for lst, tg in ((uh, "uh"), (ul, "ul"), (u2h, "u2h"),
                                    (u2l, "u2l")):
                        t_ = work.tile([128, 384], BF16, tag=f"{tg}{pt}",
                                       bufs=1)[:, :C8]
                        v.memset(t_[:], 0)
                        lst.append(t_)
                    continue
                pu = pb.tile([128, C8], F32, tag="pb")
                for bi, (fb, rows) in enumerate(fblocks):
                    te.matmul(pu[:], fb[:, pt * 128:(pt + 1) * 128],
                              wa_blk[bi][:], start=(bi == 0),
                              stop=(bi == len(fblocks) - 1))
                h_ = work.tile([128, 384], BF16, tag=f"uh{pt}",
                               bufs=1)[:, :C8]
                sc.copy(h_[:], pu[:])
                l_ = work.tile([128, 384], BF16, tag=f"ul{pt}",
                               bufs=1)[:, :C8]
                v.scalar_tensor_tensor(l_[:], pu[:], 1.0, h_[:],
                                       OP.mult, OP.subtract)
                sqf = work.tile([128, 384], F32, tag="sqf")[:, :C8]
                sc.square(sqf[:], pu[:])
                if li == 0:
                    h2 = work.tile([128, 384], BF16, tag=f"u2h{pt}",
                                   bufs=1)[:, :C8]
                    sc.copy(h2[:], sqf[:])
                    l2 = work.tile([128, 384], BF16, tag=f"u2l{pt}",
                                   bufs=1)[:, :C8]
                    v.scalar_tensor_tensor(l2[:], sqf[:], 1.0, h2[:],
                                           OP.mult, OP.subtract)
                    u2h.append(h2)
                    u2l.append(l2)
                else:
                    # total-only S2: sum_m cnt(m)*U^2(c,m), 1-col matmuls
                    for ct in range(NCT):
                        cw = min(128, C8 - ct * 128)
                        cs = ct * 128
                        p2 = pa.tile([cw, 512], F32, tag="pa")[:, :1]
                        te.matmul(p2[:], sqf[:, cs:cs + cw],
                                  CNT[pt // 2][pt % 2][:],
                                  start=True, stop=True)
                        v.tensor_tensor(s2acc[ct][:], s2acc[ct][:], p2[:],
                                        OP.add)
                uh.append(h_)
                ul.append(l_)

            # V (f32), wd streamed
            V = []
            for ct in range(NCT):
                cw = min(128, C8 - ct * 128)
                vt = work.tile([cw, PTS], F32, tag=f"V{ct}", bufs=1)
                if "v" in ABLATE:
                    v.memset(vt[:], 0)
                    V.append(vt)
                    continue
                for ch in range(2):
                    pv = pa.tile([cw, 512], F32, tag="pa")
                    ro = 0
                    for bi, (fb, rows) in enumerate(fblocks):
                        wd = work.tile([rows, 128], F32, tag="wdblk",
                                       bufs=3)[:, :cw]
                        sy.dma_start(wd[:], io[f"wd{li}"].ap()[
                            ro:ro + rows, ct * 128:ct * 128 + cw])
                        te.matmul(pv[:], wd[:],
                                  fb[:, ch * 512:(ch + 1) * 512],
                                  start=(bi == 0),
                                  stop=(bi == len(fblocks) - 1))
                        ro += rows
                    sc.copy(vt[:, ch * 512:(ch + 1) * 512], pv[:])
                V.append(vt)

            racc_s = [work.tile([min(128, C8 - ct * 128), BT], F32,
                                tag=f"rs{ct}") for ct in range(NCT)]
            racc_s2 = [work.tile([min(128, C8 - ct * 128), BT], F32,
                                 tag=f"rs2{ct}") for ct in range(NCT)]
            racc_vs = [work.tile([min(128, C8 - ct * 128), BT], F32,
                                 tag=f"rvs{ct}") for ct in range(NCT)]
            upool = [work.tile([min(128, C8 - ct * 128), PTS], F32,
                               tag=f"up{ct}", bufs=1) for ct in range(NCT)]
            if "pool" in ABLATE:
                for u_ in upool:
                    v.memset(u_[:], 0)
            if "stats" in ABLATE:
                for r_ in racc_s + racc_s2 + racc_vs:
                    v.memset(r_[:], 0)

            for f in range(BT):
                gts = build_G(f, li == 0) if "g" not in ABLATE else None
                if li == 0 and "g" not in ABLATE:
                    ats = []
                    with nc.allow_low_precision(reason="0/1 adjacency"):
                        for t in range(2):
                            at = pers.tile([128, N], BF16, tag=f"A{f}_{t}")
                            v.tensor_reduce(at[:], gts[t][:].rearrange(
                                "p (n k) -> p n k", k=K), axis=AX, op=OP.add)
                            ats.append(at)
                    A4[f] = ats
                    for t in range(2):
                        cn = pers.tile([128, 1], F32, tag=f"cnt{f}_{t}")
                        v.tensor_reduce(cn[:], A4[f][t][:], axis=AX,
                                        op=OP.add)
                        CNT[f][t] = cn
                for ct in range(NCT):
                    cw = min(128, C8 - ct * 128)
                    cs = ct * 128
                    for c8 in range(NCHK if "exp" not in ABLATE else 0):
                        px = pe.tile([cw, CHK * K], F32, tag="pe")
                        for sub0, sub1 in ((0, 512), (512, CHK * K)):
                            ops = [(0, uh), (1, uh)]
                            for oi, (mt, usrc) in enumerate(ops):
                                te.matmul(
                                    px[:, sub0:sub1],
                                    usrc[2 * f + mt][:, cs:cs + cw],
                                    gts[mt][:, c8 * CHK * K + sub0:
                                            c8 * CHK * K + sub1],
                                    start=(oi == 0), stop=(oi == 1))
                        if "pool" not in ABLATE:
                            v.tensor_reduce(
                                upool[ct][:, f * N + c8 * CHK:
                                          f * N + (c8 + 1) * CHK],
                                px[:].rearrange("p (n k) -> p n k", k=K),
                                axis=AX, op=OP.max)
                    if "stats" in ABLATE:
                        continue
                    ps1 = pa.tile([cw, N], F32, tag="pa")
                    ops = [(0, uh), (0, ul), (1, uh), (1, ul)]
                    for oi, (mt, usrc) in enumerate(ops):
                        te.matmul(ps1[:], usrc[2 * f + mt][:, cs:cs + cw],
                                  A4[f][mt][:], start=(oi == 0),
                                  stop=(oi == 3))
                    v.tensor_reduce(racc_s[ct][:, f:f + 1], ps1[:],
                                    axis=AX, op=OP.add)
                    scr = work.tile([cw, N], F32, tag="scr")
                    v.scalar_tensor_tensor(
                        scr[:], V[ct][:, f * N:(f + 1) * N], 1.0, ps1[:],
                        OP.mult, OP.mult, accum_out=racc_vs[ct][:, f:f + 1])
                    if li == 0:
                        ps2 = pa.tile([cw, N], F32, tag="pa")
                        ops = [(0, u2h), (0, u2l), (1, u2h), (1, u2l)]
                        for oi, (mt, usrc) in enumerate(ops):
                            te.matmul(ps2[:],
                                      usrc[2 * f + mt][:, cs:cs + cw],
                                      A4[f][mt][:], start=(oi == 0),
                                      stop=(oi == 3))
                        v.tensor_reduce(racc_s2[ct][:, f:f + 1], ps2[:],
                                        axis=AX, op=OP.add)

            # finalize stats + BN + lrelu + tp; write comb to DRAM
            for ct in range(NCT):
                cw = min(128, C8 - ct * 128)
                cs = ct * 128
                col = lambda tag: work.tile([cw, 1], F32, tag=tag)
                rs, rs2, rvs = col("c_rs"), col("c_rs2"), col("c_rvs")
                v.tensor_reduce(rs[:], racc_s[ct][:], axis=AX, op=OP.add)
                if li == 0:
                    v.tensor_reduce(rs2[:], racc_s2[ct][:], axis=AX,
                                    op=OP.add)
                else:
                    rs2 = s2acc[ct]
                v.tensor_reduce(rvs[:], racc_vs[ct][:], axis=AX, op=OP.add)
                rv, rv2 = col("c_rv"), col("c_rv2")
                scrv = work.tile([cw, PTS], F32, tag="hwork")
                v.tensor_scalar(scrv[:], V[ct][:], 1.0, 0.0, OP.mult,
                                OP.add, accum_out=rv[:])
                v.scalar_tensor_tensor(scrv[:], V[ct][:], 1.0, V[ct][:],
                                       OP.mult, OP.mult, accum_out=rv2[:])
                sum_h, t1, sum_h2 = col("c_sh"), col("c_t1"), col("c_sh2")
                v.scalar_tensor_tensor(sum_h[:], rv[:], float(K), rs[:],
                                       OP.mult, OP.add)
                v.scalar_tensor_tensor(t1[:], rvs[:], 2.0, rs2[:],
                                       OP.mult, OP.add)
                v.scalar_tensor_tensor(sum_h2[:], rv2[:], float(K), t1[:],
                                       OP.mult, OP.add)
                mean, ex2, var = col("c_mean"), col("c_ex2"), col("c_var")
                v.tensor_scalar(mean[:], sum_h[:], 1.0 / MBN, None, OP.mult)
                v.tensor_scalar(ex2[:], sum_h2[:], 1.0 / MBN, None, OP.mult)
                m2 = col("c_m2")
                v.tensor_tensor(m2[:], mean[:], mean[:], OP.mult)
                v.tensor_tensor(var[:], ex2[:], m2[:], OP.subtract)
                std, inv, scal, bias2 = (col("c_std"), col("c_inv"),
                                         col("c_scal"), col("c_bias"))
                sc.activation(std[:], var[:], AF.Sqrt, bias=eps_col[:cw])
                v.reciprocal(inv[:], std[:])
                gcol = work.tile([cw, 1], F32, tag="gcol")
                sy.dma_start(gcol[:], io[f"g{li}"].ap()[cs:cs + cw])
                bcol = work.tile([cw, 1], F32, tag="bcol")
                sy.dma_start(bcol[:], io[f"b{li}"].ap()[cs:cs + cw])
                v.tensor_tensor(scal[:], inv[:], gcol[:], OP.mult)
                tm = col("c_tm")
                v.tensor_tensor(tm[:], mean[:], scal[:], OP.mult)
                v.tensor_tensor(bias2[:], bcol[:], tm[:], OP.subtract)
                h = work.tile([cw, PTS], F32, tag="hwork")
                v.tensor_tensor(h[:], upool[ct][:], V[ct][:], OP.add)
                bn = work.tile([cw, PTS], F32, tag="hwork")
                v.tensor_scalar(bn[:], h[:], scal[:], bias2[:],
                                OP.mult, OP.add)
                hpost = work.tile([cw, PTS], F32, tag="hwork")
                v.scalar_tensor_tensor(hpost[:], bn[:], 0.2, bn[:],
                                       OP.mult, OP.max)
                tps = work.tile([cw, N], F32, tag="tps")
                v.tensor_tensor(tps[:], hpost[:, 0:N], hpost[:, N:2 * N],
                                OP.add)
                tpx = work.tile([cw, N], F32, tag="tpx")
                v.tensor_scalar(tpx[:], tps[:], 0.5, None, OP.mult)
                # write [hp; tp] to the layer's comb DRAM buffer
                cd = (comb0_d, ag1_in, ag2_in, comb3_d)[li]
                sy.dma_start(cd.ap()[cs:cs + cw], hpost[:])
                for f in range(BT):
                    sy.dma_start(
                        cd.ap()[C8 + cs:C8 + cs + cw,
                                f * N:(f + 1) * N], tpx[:])
            if li == 1:
                gp.collective_compute("AllGather", OP.bypass,
                                      replica_groups=rg,
                                      ins=[ag1_in.ap()], outs=[ag1_out.ap()])
            elif li == 2:
                gp.collective_compute("AllGather", OP.bypass,
                                      replica_groups=rg,
                                      ins=[ag2_in.ap()], outs=[ag2_out.ap()])

        # ---- L4 ----
        pieces = [(comb0_d, 0, 48), (comb0_d, 48, 48),
                  (ag1_in, 0, 24), (ag1_in, 24, 24),
                  (ag2_in, 0, 96), (ag2_in, 96, 96)]
        for ct in range(3):
            pieces.append((comb3_d, ct * 128, 128))
        for ct in range(3):
            pieces.append((comb3_d, 384 + ct * 128, 128))

        for ch in range(2):
            if "l4" in ABLATE:
                for half in range(2):
                    yp = work.tile([128, 512], F32, tag="ypc")
                    v.memset(yp[:], 0)
                    sy.dma_start(ar_in.ap()[half * 128:(half + 1) * 128,
                                            ch * 512:(ch + 1) * 512], yp[:])
                continue
            pys = [pa.tile([128, 512], F32, tag="pa") for _ in range(2)]
            ro = 0
            for bi, (src, off, rows) in enumerate(pieces):
                pc = work.tile([128, 512], F32, tag="l4p", bufs=3)[:rows, :]
                sy.dma_start(pc[:], src.ap()[off:off + rows,
                                             ch * 512:(ch + 1) * 512])
                wb = work.tile([128, 256], F32, tag="l4w", bufs=3)[:rows, :]
                sy.dma_start(wb[:], io["w4t"].ap()[ro:ro + rows])
                for half in range(2):
                    te.matmul(pys[half][:],
                              wb[:, half * 128:(half + 1) * 128], pc[:],
                              start=(bi == 0), stop=(bi == len(pieces) - 1))
                ro += rows
            for half in range(2):
                yp = work.tile([128, 512], F32, tag="ypc")
                v.tensor_copy(yp[:], pys[half][:])
                sy.dma_start(ar_in.ap()[half * 128:(half + 1) * 128,
                                        ch * 512:(ch + 1) * 512], yp[:])
        gp.collective_compute("AllReduce", OP.add, replica_groups=rg,
                              ins=[ar_in.ap()], outs=[ar_out.ap()])
        for half in range(2):
            yf = work.tile([128, PTS], F32, tag="hwork")
            sy.dma_start(yf[:], ar_out.ap()[half * 128:(half + 1) * 128])
            col = lambda tag: work.tile([128, 1], F32, tag=tag)
            ry, ry2 = col("y_r"), col("y_r2")
            scr = work.tile([128, PTS], F32, tag="hwork")
            v.tensor_scalar(scr[:], yf[:], 1.0, 0.0, OP.mult, OP.add,
                            accum_out=ry[:])
            v.scalar_tensor_tensor(scr[:], yf[:], 1.0, yf[:], OP.mult,
                                   OP.mult, accum_out=ry2[:])
            mean, ex2, var = col("y_mean"), col("y_ex2"), col("y_var")
            v.tensor_scalar(mean[:], ry[:], 1.0 / PTS, None, OP.mult)
            v.tensor_scalar(ex2[:], ry2[:], 1.0 / PTS, None, OP.mult)
            m2 = col("y_m2")
            v.tensor_tensor(m2[:], mean[:], mean[:], OP.mult)
            v.tensor_tensor(var[:], ex2[:], m2[:], OP.subtract)
            std, inv, scal, bias2 = (col("y_std"), col("y_inv"),
                                     col("y_scal"), col("y_bias"))
            sc.activation(std[:], var[:], AF.Sqrt, bias=eps_col[:])
            v.reciprocal(inv[:], std[:])
            gcol = work.tile([128, 1], F32, tag="y_g")
            sy.dma_start(gcol[:], io["g4"].ap()[half * 128:(half + 1) * 128])
            bcol = work.tile([128, 1], F32, tag="y_b")
            sy.dma_start(bcol[:], io["b4"].ap()[half * 128:(half + 1) * 128])
            v.tensor_tensor(scal[:], inv[:], gcol[:], OP.mult)
            tm = col("y_tm")
            v.tensor_tensor(tm[:], mean[:], scal[:], OP.mult)
            v.tensor_tensor(bias2[:], bcol[:], tm[:], OP.subtract)
            bn = work.tile([128, PTS], F32, tag="hwork")
            v.tensor_scalar(bn[:], yf[:], scal[:], bias2[:], OP.mult, OP.add)
            yo = work.tile([128, PTS], F32, tag="hwork")
            v.scalar_tensor_tensor(yo[:], bn[:], 0.2, bn[:], OP.mult, OP.max)
            dst = out_d.ap().rearrange("b t c n -> c (b t) n")
            sy.dma_start(dst[half * 128:(half + 1) * 128],
                         yo[:].rearrange("p (f n) -> p f n", n=N))

        for p in (pb, pa, pe, work, pers):
            p.release()

    nc.compile()
    return nc


def _perm_for(C8):
    C = C8 * NCORE
    out = []
    for r in range(NCORE):
        out += list(range(r * C8, (r + 1) * C8))
        out += list(range(C + r * C8, C + (r + 1) * C8))
    return np.array(out)


def _prep_inputs(inputs):
    x = np.asarray(inputs["x"], np.float32)
    flat = x.reshape(BT, N, 3)
    x_t = np.ascontiguousarray(flat.transpose(0, 2, 1))
    iota = np.arange(256, dtype=np.float32).reshape(256, 1)
    negones = np.full((3, 128), -1.0, np.float32)

    base = {
        "x_t": x_t, "iota": iota, "negones": negones,
        "g4": np.asarray(inputs["g4"], np.float32).reshape(256, 1),
        "b4": np.asarray(inputs["b4"], np.float32).reshape(256, 1),
    }
    perm = np.arange(3)
    in_maps = [dict(base) for _ in range(NCORE)]
    w4 = np.asarray(inputs["w4"], np.float32)
    for li in range(4):
        C = COUT[li]
        F = FIN[li]
        W = np.asarray(inputs[f"w{li}"], np.float32)
        Wa_full = W[:, :F][:, perm]
        Wd_full = (W[:, F:] - W[:, :F])[:, perm]
        g = np.asarray(inputs[f"g{li}"], np.float32)
        b = np.asarray(inputs[f"b{li}"], np.float32)
        for r in range(NCORE):
            if li == 0:
                rows = slice(0, 48)
            else:
                rows = slice(r * SL[li], (r + 1) * SL[li])
            in_maps[r][f"wa{li}"] = np.ascontiguousarray(Wa_full[rows].T)
            in_maps[r][f"wd{li}"] = np.ascontiguousarray(Wd_full[rows].T)
            in_maps[r][f"g{li}"] = g[rows].reshape(-1, 1).copy()
            in_maps[r][f"b{li}"] = b[rows].reshape(-1, 1).copy()
        # L0 replicated: f1 = [hp0; tp0] stays in reference order.
        perm = np.arange(2 * C) if li == 0 else _perm_for(C // NCORE)
    for r in range(NCORE):
        cols = []
        cols.append((np.arange(0, 48), 1 / 8))
        cols.append((np.arange(48, 96), 1 / 8))
        cols.append((96 + np.arange(r * 24, (r + 1) * 24), 1.0))
        cols.append((288 + np.arange(r * 24, (r + 1) * 24), 1.0))
        cols.append((480 + np.arange(r * 96, (r + 1) * 96), 1.0))
        cols.append((1248 + np.arange(r * 96, (r + 1) * 96), 1.0))
        cols.append((2016 + np.arange(r * 384, (r + 1) * 384), 1.0))
        cols.append((5088 + np.arange(r * 384, (r + 1) * 384), 1.0))
        blocks = [np.ascontiguousarray(w4[:, c].T) * s for c, s in cols]
        in_maps[r]["w4t"] = np.concatenate(blocks, axis=0).astype(np.float32)
    return in_maps


_NC_CACHE = []


def kernel(**inputs):
    if not _NC_CACHE:
        _NC_CACHE.append(build_nc())
    nc = _NC_CACHE[0]
    in_maps = _prep_inputs(inputs)
    res = run_bass_kernel_spmd(nc, in_maps, list(range(NCORE)))
    return np.asarray(res.results[0]["out"])

